# revision 6
# baseline (speedup 1.0000x reference)
"""GCN layer (SpMM) Trainium2 kernel: out = segment_sum(vals * x[cols], rows).

Self-contained: host-side sharding/preprocessing + a uniform Bass/Tile
program run SPMD on 8 NeuronCores via bass_utils.run_bass_kernel_spmd.

Design (row-partition SpMM, 8-way graph parallel):
  - adj_rows is sorted; each core takes a contiguous 1/8 row range.
  - Rows are packed into groups of <=40 rows by first-fit-decreasing so
    each (group, chunk) holds <=128 edge tokens; CH=4 column chunks keep
    SWDGE gather indices within int16.
  - x is stored bf16 padded to 128 cols (256B rows). Neighbor rows are
    fetched per edge with SWDGE dma_gather: one 1024-descriptor call per
    (sg of 8 groups, chunk), rotated across the 4 SWDGE queues; all idx
    data is preloaded to SBUF once.
  - Reduction per group on-chip: DVE builds a fused scaled one-hot
    S[tok, w] = val * (iota[w] == rel[tok]) in bf16 (two batched ops per
    sg over [128, 32, 40]); PE accumulates S^T @ tok into a PSUM [40,64]
    fp32 tile (4 matmuls per group); Act stages PSUM->SBUF; the staged
    rows stream out with one contiguous DMA per sg.
  - Host scatters staged group rows back to out[row] (fp32 accumulate
    precision; bf16 inputs keep rel err ~2.5e-3, well under 2e-2).
"""
import numpy as np
import ml_dtypes

D = 64
DP = 128  # padded bf16 x row: 128 cols = 256B
P = 128
N_CORES = 8
CH = 4
TPC = 1
SG = 8
CAP_ROWS = 40
SPAN = 1  # sgs per gather call span
N_QUEUES = 4


def _ffd_pack(deg, cap_rows, seg_cap):
    """First-fit-decreasing row->group packing: per-chunk caps of seg_cap
    tokens, <= cap_rows rows per group."""
    n_rows, n_ch = deg.shape
    order = np.argsort(-deg.max(1), kind="stable")
    caps = np.zeros((0, n_ch), np.int64)
    slots = np.zeros((0,), np.int64)
    gid = np.zeros(n_rows, np.int64)
    rank = np.zeros(n_rows, np.int64)
    for r in order:
        d = deg[r]
        ok = np.nonzero(((caps + d) <= seg_cap).all(1) & (slots < cap_rows))[0]
        if ok.size:
            g = ok[0]
        else:
            g = caps.shape[0]
            caps = np.vstack([caps, np.zeros((1, n_ch), np.int64)])
            slots = np.append(slots, 0)
        gid[r] = g
        rank[r] = slots[g]
        caps[g] += d
        slots[g] += 1
    return gid, rank, caps.shape[0]



def _pack_core(rows, cols, vals, r_lo, r_hi, G, chunk_rows):
    seg_cap = TPC * P  # 128
    e_lo = np.searchsorted(rows, r_lo, "left")
    e_hi = np.searchsorted(rows, r_hi, "left")
    r = rows[e_lo:e_hi].astype(np.int64)
    c = cols[e_lo:e_hi].astype(np.int64)
    v = vals[e_lo:e_hi].astype(np.float32)
    ch = c // chunk_rows
    n_rows_core = r_hi - r_lo
    rr = r - r_lo
    deg = np.zeros((n_rows_core, CH), np.int64)
    for cc in range(CH):
        deg[:, cc] = np.bincount(rr[ch == cc], minlength=n_rows_core)
    gid, rank, n_groups_real = _ffd_pack(deg, CAP_ROWS, seg_cap)
    assert n_groups_real <= G, (n_groups_real, G)

    order = np.argsort(ch, kind="stable")
    r_s, c_s, v_s, ch_s = rr[order], c[order], v[order], ch[order]
    chunk_lo = np.searchsorted(ch_s, np.arange(CH), "left")
    chunk_hi = np.searchsorted(ch_s, np.arange(CH), "right")

    idx_lin = np.zeros((G, CH, seg_cap), np.int16)
    rel_lin = np.zeros((G, CH, seg_cap), np.float32)
    val_lin = np.zeros((G, CH, seg_cap), np.float32)
    row_of = np.full((G, CAP_ROWS), -1, np.int64)
    row_of[gid, rank] = r_lo + np.arange(n_rows_core)

    for cc in range(CH):
        lo, hi = chunk_lo[cc], chunk_hi[cc]
        rcc = r_s[lo:hi]
        icc = (c_s[lo:hi] - cc * chunk_rows).astype(np.int16)
        vcc = v_s[lo:hi]
        relcc = rank[rcc].astype(np.float32)
        gcc = gid[rcc]
        o2 = np.argsort(gcc, kind="stable")
        gcc_s = gcc[o2]
        grp_start = np.searchsorted(gcc_s, gcc_s, "left")
        pos = np.arange(hi - lo) - grp_start
        idx_lin[gcc_s, cc, pos] = icc[o2]
        rel_lin[gcc_s, cc, pos] = relcc[o2]
        val_lin[gcc_s, cc, pos] = vcc[o2]

    ncol = G * CH  # one column tile per (group, chunk)
    # columns (g, cc); padding tokens rel=-1 (eq -> 0) and val=0
    pad = val_lin[:, :, :] == 0.0
    rel_masked = np.where(pad, -1.0, rel_lin)
    rel_all = np.transpose(rel_masked, (2, 0, 1)).reshape(P, ncol)
    val_all = np.transpose(val_lin, (2, 0, 1)).reshape(P, ncol)

    # idx: one call per (span, cc) = SPAN*SG groups x 128 tokens
    n_span = G // (SPAN * SG)
    call_tok = SPAN * SG * seg_cap
    ccols = call_tok // 16
    idx_all = np.zeros((P, n_span * CH * ccols), np.int16)
    for sp in range(n_span):
        g0 = sp * SPAN * SG
        for cc in range(CH):
            lin = idx_lin[g0 : g0 + SPAN * SG, cc, :].reshape(-1)
            blk = lin.reshape(ccols, 16).T
            col0 = (sp * CH + cc) * ccols
            idx_all[:, col0 : col0 + ccols] = np.tile(blk, (P // 16, 1))

    iota = np.broadcast_to(np.arange(CAP_ROWS, dtype=np.float32), (P, CAP_ROWS))
    meta = np.ascontiguousarray(
        np.concatenate([iota, rel_all, val_all], 1)
    ).astype(ml_dtypes.bfloat16)
    return idx_all, meta, row_of


def _count_groups(rows, cols, r_lo, r_hi, chunk_rows):
    seg_cap = TPC * P
    e_lo = np.searchsorted(rows, r_lo, "left")
    e_hi = np.searchsorted(rows, r_hi, "left")
    r = rows[e_lo:e_hi].astype(np.int64) - r_lo
    c = cols[e_lo:e_hi].astype(np.int64)
    ch = c // chunk_rows
    n_rows_core = r_hi - r_lo
    deg = np.zeros((n_rows_core, CH), np.int64)
    for cc in range(CH):
        deg[:, cc] = np.bincount(r[ch == cc], minlength=n_rows_core)
    _, _, n = _ffd_pack(deg, CAP_ROWS, seg_cap)
    return n


def _build_program(n_x_rows_padded, G, repeats=1):
    import concourse.bacc as bacc
    import concourse.mybir as mybir
    import concourse.tile as tile

    seg_cap = TPC * P
    n_sg = G // SG
    n_span = G // (SPAN * SG)
    call_tok = SPAN * SG * seg_cap
    ccols = call_tok // 16
    ncol = G * CH
    chunk_rows = n_x_rows_padded // CH

    nc = bacc.Bacc(None, num_swdge_queues=N_QUEUES)
    x_t = nc.dram_tensor("x", [n_x_rows_padded, DP], mybir.dt.bfloat16,
                         kind="ExternalInput")
    idx_t = nc.dram_tensor("idx", [P, n_span * CH * ccols], mybir.dt.int16,
                           kind="ExternalInput")
    meta_t = nc.dram_tensor("meta", [P, CAP_ROWS + 2 * ncol],
                            mybir.dt.bfloat16, kind="ExternalInput")
    out_t = nc.dram_tensor("out", [CAP_ROWS, G * D], mybir.dt.float32,
                           kind="ExternalOutput")

    with tile.TileContext(nc) as tc:
        with (
            tc.tile_pool(name="const", bufs=1) as const_pool,
            tc.tile_pool(name="tokp", bufs=3) as tok_pool,
            tc.tile_pool(name="stagep", bufs=3) as stage_pool,
            tc.tile_pool(name="work", bufs=6) as work_pool,
            tc.tile_pool(name="psum", bufs=8, space="PSUM") as psum_pool,
        ):
            meta_sb = const_pool.tile([P, CAP_ROWS + 2 * ncol],
                                      mybir.dt.bfloat16, tag="meta")
            nc.sync.dma_start(meta_sb[:], meta_t[:])
            idx_sb_all = const_pool.tile([P, n_span * CH * ccols],
                                         mybir.dt.int16, tag="idxall")
            nc.sync.dma_start(idx_sb_all[:], idx_t[:])
            iota_f = meta_sb[:, 0:CAP_ROWS]
            rel_all = meta_sb[:, CAP_ROWS : CAP_ROWS + ncol]
            val_all = meta_sb[:, CAP_ROWS + ncol : CAP_ROWS + 2 * ncol]

            for rep in range(repeats):
                for sp in range(n_span):
                    idx_sb = idx_sb_all[:, sp * CH * ccols :
                                        (sp + 1) * CH * ccols]
                    toks = []
                    for cc in range(CH):
                        tok = tok_pool.tile([P, SPAN * SG, DP],
                                            mybir.dt.bfloat16, tag=f"tok{cc}")
                        nc.gpsimd.dma_gather(
                            tok[:],
                            x_t[cc * chunk_rows : (cc + 1) * chunk_rows, :],
                            idx_sb[:, cc * ccols : (cc + 1) * ccols],
                            call_tok,
                            call_tok,
                            DP,
                            single_packet=False,
                            queue_num=(cc + sp) % N_QUEUES,
                        )
                        toks.append(tok)
                    for sl in range(SPAN):
                        sg = sp * SPAN + sl
                        k0 = sg * SG * CH
                        S = work_pool.tile([P, SG * CH, CAP_ROWS],
                                           mybir.dt.bfloat16, tag="S")
                        nc.vector.tensor_tensor(
                            out=S[:],
                            in0=iota_f.unsqueeze(1)
                            .broadcast_to([P, SG * CH, CAP_ROWS]),
                            in1=rel_all[:, k0 : k0 + SG * CH]
                            .unsqueeze(2)
                            .broadcast_to([P, SG * CH, CAP_ROWS]),
                            op=mybir.AluOpType.is_equal,
                        )
                        nc.vector.tensor_tensor(
                            out=S[:],
                            in0=S[:],
                            in1=val_all[:, k0 : k0 + SG * CH]
                            .unsqueeze(2)
                            .broadcast_to([P, SG * CH, CAP_ROWS]),
                            op=mybir.AluOpType.mult,
                        )
                        stage = stage_pool.tile([P, SG * D], mybir.dt.float32,
                                                tag="stage")
                        for dg in range(SG):
                            acc = psum_pool.tile([CAP_ROWS, D],
                                                 mybir.dt.float32, tag="acc")
                            for cc in range(CH):
                                nc.tensor.matmul(
                                    acc[:],
                                    S[:, dg * CH + cc, :],
                                    toks[cc][:, sl * SG + dg, 0:D],
                                    start=(cc == 0), stop=(cc == CH - 1),
                                )
                            nc.scalar.copy(
                                stage[:CAP_ROWS, dg * D : (dg + 1) * D],
                                acc[:],
                            )
                        g0 = sg * SG
                        nc.sync.dma_start(
                            out_t[0:CAP_ROWS, g0 * D : (g0 + SG) * D],
                            stage[:CAP_ROWS, :],
                        )
    nc.compile()
    return nc


def _legalize_waits(nc):
    import concourse.mybir as mybir

    for f in nc.m.functions:
        for blk in f.blocks:
            newlist = []
            for ins in blk.instructions:
                si = ins.sync_info
                ow = list(si.on_wait) if si else []
                if len(ow) > 1:
                    for i, w in enumerate(ow[:-1]):
                        nop = mybir.InstNoOp(name=f"{ins.name}_ws{i}", ins=[],
                                             outs=[])
                        nop.engine = ins.engine
                        nop.sync_info = mybir.SyncInfo(on_wait=[w], on_update=[])
                        newlist.append(nop)
                    ins.sync_info = mybir.SyncInfo(
                        on_wait=[ow[-1]], on_update=list(si.on_update)
                    )
                newlist.append(ins)
            blk.instructions[:] = newlist


_LAST_RESULTS = None
_PROG_CACHE = {}


def prepare(adj_rows, adj_cols, adj_vals, x, repeats=1):
    global TPC, CAP_ROWS
    rows = np.asarray(adj_rows).astype(np.int64)
    cols = np.asarray(adj_cols).astype(np.int64)
    vals = np.asarray(adj_vals).astype(np.float32)
    xf = np.ascontiguousarray(np.asarray(x), dtype=np.float32)
    n_nodes = xf.shape[0]
    chunk_rows = -(-n_nodes // CH)
    n_x_pad = chunk_rows * CH
    xpad = np.zeros((n_x_pad, DP), np.float32)
    xpad[:n_nodes, :D] = xf
    x_bf = xpad.astype(ml_dtypes.bfloat16)

    bounds = [round(i * n_nodes / N_CORES) for i in range(N_CORES + 1)]
    G = 0
    for i in range(N_CORES):
        G = max(G, _count_groups(rows, cols, bounds[i], bounds[i + 1],
                                 chunk_rows))
    gq = SPAN * SG
    G = -(-G // gq) * gq

    in_maps = []
    row_ofs = []
    for i in range(N_CORES):
        idx_all, meta, row_of = _pack_core(
            rows, cols, vals, bounds[i], bounds[i + 1], G, chunk_rows
        )
        in_maps.append({"x": x_bf, "idx": idx_all, "meta": meta})
        row_ofs.append(row_of)

    key = (G, n_x_pad, repeats)
    nc = _PROG_CACHE.get(key)
    if nc is None:
        nc = _build_program(n_x_pad, G, repeats=repeats)
        _legalize_waits(nc)
        _PROG_CACHE[key] = nc
    return nc, in_maps, row_ofs, n_nodes, G


def _unshard(results, row_ofs, n_nodes, G):
    out = np.zeros((n_nodes, D), np.float32)
    for i in range(N_CORES):
        staged = results[i]["out"].reshape(CAP_ROWS, G, D).transpose(1, 0, 2)
        row_of = row_ofs[i]
        mask = row_of >= 0
        out[row_of[mask]] = staged[mask]
    return out


def kernel(adj_rows, adj_cols, adj_vals, x):
    global _LAST_RESULTS
    from concourse.bass_utils import run_bass_kernel_spmd

    nc, in_maps, row_ofs, n_nodes, G = prepare(adj_rows, adj_cols, adj_vals, x)
    res = run_bass_kernel_spmd(nc, in_maps, core_ids=list(range(N_CORES)))
    _LAST_RESULTS = res
    return _unshard(res.results, row_ofs, n_nodes, G)
